# revision 1
# baseline (speedup 1.0000x reference)
"""Trainium2 Bass kernel for supervised contrastive loss (8-core SPMD).

Math (per reference):
    f = x / max(||x||, 1e-12)            row-normalized features  [B, D]
    s = (f f^T) / TEMP                                            [B, B]
    E = exp(s) with diag zeroed
    P_i = sum_{j != i, l_j == l_i} E_ij   (positives)
    T_i = sum_{j != i} E_ij               (positives + negatives)
    loss = mean_i [ log(T_i + EPS) - log(P_i) ]

Distribution: row-block shard. Core c owns rows m in [1024c, 1024(c+1)).
Each core computes E^T blocks [j-chunk(128) x m(1024)] with j on the
partition dim, so BOTH reductions (over j) are partition-contractions and
run on the TensorEngine:
    PS1[c', m] = sum_j Y'[j, c'] * E[j, m]     (Y' = one-hot(labels) | ones)
row 100 of PS1 = T_m, and P_m = PS1[l_m, m] (recovered with a one-hot
mask + ones-matmul). Per-core scalar partial losses are summed on host.

SPMD uniformity: every core runs the identical program; per-core variation
lives entirely in the input data. Chunk arrays are rotated per core so the
core's own (diagonal) chunks are always processed at t = 0..7.
"""

import numpy as np
import ml_dtypes

TEMPERATURE = 0.07
EPS = 1e-8
B = 8192
D = 512
NCORES = 8
M = B // NCORES          # 1024 rows per core
NCH = B // 128           # 64 j-chunks of 128
BCH = M // 128           # 8 chunks belonging to the core's own block
NCLS = 100               # label classes
YC = NCLS + 1            # one-hot columns + ones column

_CACHE = {}


def _build_bass():
    import concourse.bass as bass
    import concourse.bacc as bacc
    import concourse.tile as tile
    from concourse import mybir
    from contextlib import ExitStack

    f32 = mybir.dt.float32
    bf16 = mybir.dt.bfloat16
    AF = mybir.ActivationFunctionType
    OP = mybir.AluOpType

    nc = bacc.Bacc()

    # ---- I/O ----------------------------------------------------------
    # xt[t, p, dc*128+jj] = x[jc_t*128+jj, dc*128+p]   (chunk-major x^T)
    xt_d = nc.declare_dram_parameter("xt", [NCH, 128, D], bf16, isOutput=False)
    # xnat[t, p, d] = x[jc_t*128+p, d]                 (natural row tiles)
    xnat_d = nc.declare_dram_parameter("xnat", [NCH, 128, D], bf16, isOutput=False)
    # all label/iota constants in ONE tensor (single DMA -> single wait for
    # downstream DVE ops, which only support one sync-wait in walrus):
    #   [:, 0:6464]        iota[p, t, c'] = c' - 1
    #   [:, 6464:12928]    labbc[p, t, c'] = labels[jc_t*128+p]
    #   [:, 12928:13952]   labblk[p, m] = labels[block row m]
    #   [:, 13952:13954]   iotap[p] = p - 1 as raw f32 (two bf16 slots)
    LC = NCH * YC
    labio_d = nc.declare_dram_parameter(
        "labio", [128, 2 * LC + M + 2], bf16, isOutput=False
    )
    loss_d = nc.declare_dram_parameter("loss", [1, 1], f32, isOutput=True)

    with ExitStack() as ctx:
        tc = ctx.enter_context(tile.TileContext(nc))
        const = ctx.enter_context(tc.tile_pool(name="const", bufs=1))
        xtp = ctx.enter_context(tc.tile_pool(name="xtp", bufs=4))
        xnp = ctx.enter_context(tc.tile_pool(name="xnp", bufs=4))
        sqp = ctx.enter_context(tc.tile_pool(name="sqp", bufs=4))
        lnp = ctx.enter_context(tc.tile_pool(name="lnp", bufs=2))
        ep = ctx.enter_context(tc.tile_pool(name="ep", bufs=3))
        psum = ctx.enter_context(tc.tile_pool(name="psum", bufs=3, space="PSUM"))
        accp = ctx.enter_context(tc.tile_pool(name="accp", bufs=1, space="PSUM"))

        # ---- constants / label machinery ------------------------------
        labio = const.tile([128, 2 * LC + M + 2], bf16)
        nc.sync.dma_start(out=labio[:], in_=labio_d[:])
        iota_cl = labio[:, 0:LC].rearrange("p (t c) -> p t c", c=YC)
        labbc_sb = labio[:, LC : 2 * LC].rearrange("p (t c) -> p t c", c=YC)
        labblk_sb = labio[:, 2 * LC : 2 * LC + M]
        iota_p = labio[:, 2 * LC + M : 2 * LC + M + 2].bitcast(f32)

        # Y'[p, t, c'] = (c'-1 == labels[j]) for c' in 1..100; col 0 = ones
        # (T-sum column at c'=0 so T lands on PSUM partition 0).
        yall = const.tile([128, NCH, YC], bf16)
        nc.vector.tensor_tensor(
            out=yall[:], in0=iota_cl, in1=labbc_sb, op=OP.is_equal
        )
        nc.vector.memset(yall[:, :, 0:1], 1.0)

        # YblkT[c', m] = (labels[block m] == c'-1)
        yblkt = const.tile([128, M], bf16)
        nc.vector.tensor_scalar(
            out=yblkt[:], in0=labblk_sb, scalar1=iota_p, scalar2=None,
            op0=OP.is_equal,
        )

        ones101 = const.tile([128, 1], f32)
        nc.vector.memset(ones101[:], 1.0)
        bias_ltemp = const.tile([128, 1], f32)
        nc.vector.memset(bias_ltemp[:], float(-np.log(TEMPERATURE)))
        bias_eps = const.tile([128, 1], f32)
        nc.vector.memset(bias_eps[:], EPS)

        # ---- row norms:  nsq[j] = sum_d x[j,d]^2  (grouped by 8 chunks) ---
        nsqg = [
            const.tile([128, BCH], f32, tag=f"nsq{g}", name=f"nsq{g}")
            for g in range(8)
        ]
        scaleg = [
            const.tile([128, BCH], f32, tag=f"scl{g}", name=f"scl{g}")
            for g in range(8)
        ]
        for t in range(NCH):
            xn_t = xnp.tile([128, D], bf16)
            nc.sync.dma_start(out=xn_t[:], in_=xnat_d[t])
            sq_t = sqp.tile([128, D], bf16)
            # square + free-dim reduce on DVE, keeping ACT free for the exps
            nc.vector.tensor_mul(out=sq_t[:], in0=xn_t[:], in1=xn_t[:])
            nc.vector.tensor_reduce(
                out=nsqg[t // BCH][:, t % BCH : t % BCH + 1], in_=sq_t[:],
                axis=mybir.AxisListType.X, op=OP.add,
            )

        # scale_j = 1/(||x_j|| * TEMP) = exp(-0.5*ln(nsq) - ln(TEMP))
        for g in range(8):
            ln_g = lnp.tile([128, BCH], f32)
            nc.scalar.activation(out=ln_g[:], in_=nsqg[g][:], func=AF.Ln)
            nc.scalar.activation(
                out=scaleg[g][:], in_=ln_g[:], func=AF.Exp,
                bias=bias_ltemp[:], scale=-0.5,
            )

        # ---- normalized own-block x^T:  xnT[d, m] = x^T[d, m] / ||x_m|| ---
        x8 = const.tile([128, BCH, D], bf16)
        nc.sync.dma_start(
            out=x8[:], in_=xt_d[0:BCH].rearrange("t p f -> p t f")
        )

        # block-row norms in ROW layout: nsq_row[0, m] = sum_{p,dc} xT[.,m]^2
        # via DVE square + 4 accumulated ones-matmul partition reductions.
        ones_bf = const.tile([128, 1], bf16)
        nc.vector.memset(ones_bf[:], 1.0)
        x8sq = const.tile([128, BCH, D], bf16)
        nc.vector.tensor_tensor(
            out=x8sq[:], in0=x8[:], in1=x8[:], op=OP.mult
        )
        nsqrow_ps = psum.tile([128, M], f32, tag="sim", name="nsqrow_ps")
        for dc in range(4):
            for h in range(2):
                nc.tensor.matmul(
                    nsqrow_ps[0:1, h * 512 : (h + 1) * 512],
                    lhsT=ones_bf[:, 0:1],
                    rhs=x8sq[:, h * 4 : (h + 1) * 4, dc * 128 : (dc + 1) * 128],
                    start=(dc == 0),
                    stop=(dc == 3),
                )
        lnrow = const.tile([1, M], f32)
        nc.scalar.activation(out=lnrow[:], in_=nsqrow_ps[0:1, :], func=AF.Ln)
        # row of 1/||x_m|| on partition 0 of a zeroed tile, then broadcast to
        # all partitions with a ones-matmul (K=128, rows 1..127 are zero).
        rowpad = const.tile([128, M], f32)
        nc.vector.memset(rowpad[:], 0.0)
        nc.scalar.activation(
            out=rowpad[0:1, :], in_=lnrow[:], func=AF.Exp, bias=0.0, scale=-0.5
        )
        ones_f = const.tile([128, 128], f32)
        nc.vector.memset(ones_f[:], 1.0)
        invnbc_ps = psum.tile([128, M], f32, tag="sim", name="invnbc_ps")
        for h in range(2):
            nc.tensor.matmul(
                invnbc_ps[:, h * 512 : (h + 1) * 512],
                lhsT=ones_f[:],
                rhs=rowpad[:, h * 512 : (h + 1) * 512],
                start=True,
                stop=True,
            )
        # DVE copy PSUM->SBUF so the xnt multiplies have a single
        # cross-engine dependency (the x8 DMA).
        invnbc = const.tile([128, M], f32)
        nc.vector.tensor_copy(out=invnbc[:], in_=invnbc_ps[:])
        xnt = const.tile([128, 4, M], bf16)
        for dc in range(4):
            nc.vector.tensor_tensor(
                out=xnt[:, dc, :].rearrange("p (t j) -> p t j", j=128),
                in0=x8[:, :, dc * 128 : (dc + 1) * 128],
                in1=invnbc[:].rearrange("p (t j) -> p t j", j=128),
                op=OP.mult,
            )

        # ---- main loop over j-chunks ----------------------------------
        ps1 = accp.tile([128, M], f32)  # row 0: T; rows 1..100: class sums
        for t in range(NCH):
            if t < BCH:
                lhs = x8[:, t, :]
            else:
                lhs_t = xtp.tile([128, D], bf16)
                nc.sync.dma_start(out=lhs_t[:], in_=xt_d[t])
                lhs = lhs_t[:]
            ps = psum.tile([128, M], f32, tag="sim")
            for dc in range(4):
                for h in range(2):
                    nc.tensor.matmul(
                        ps[:, h * 512 : (h + 1) * 512],
                        lhsT=lhs[:, dc * 128 : (dc + 1) * 128],
                        rhs=xnt[:, dc, h * 512 : (h + 1) * 512],
                        start=(dc == 0),
                        stop=(dc == 3),
                    )
            e_t = ep.tile([128, M], bf16)
            g, k = t // BCH, t % BCH
            nc.scalar.activation(
                out=e_t[:], in_=ps[:], func=AF.Exp, scale=scaleg[g][:, k : k + 1]
            )
            if t < BCH:
                # zero the diagonal: kill (p, m) where m - p - 128*t == 0
                nc.gpsimd.affine_select(
                    out=e_t[:], in_=e_t[:], pattern=[[1, M]],
                    compare_op=OP.not_equal, fill=0.0,
                    base=-(t * 128), channel_multiplier=-1,
                )
            for h in range(2):
                nc.tensor.matmul(
                    ps1[0:YC, h * 512 : (h + 1) * 512],
                    lhsT=yall[:, t, :],
                    rhs=e_t[:, h * 512 : (h + 1) * 512],
                    start=(t == 0),
                    stop=(t == NCH - 1),
                )

        # ---- finalize: P via one-hot mask + partition reduce ----------
        maskd = const.tile([128, M], f32)
        nc.vector.tensor_tensor(
            out=maskd[0:YC, :], in0=ps1[0:YC, :], in1=yblkt[0:YC, :], op=OP.mult
        )
        pps = psum.tile([128, M], f32, tag="sim")
        for h in range(2):
            nc.tensor.matmul(
                pps[0:1, h * 512 : (h + 1) * 512],
                lhsT=ones101[0:YC, 0:1],
                rhs=maskd[0:YC, h * 512 : (h + 1) * 512],
                start=True,
                stop=True,
            )
        ln_t = const.tile([1, M], f32)
        nc.scalar.activation(
            out=ln_t[:], in_=ps1[0:1, :], func=AF.Ln, bias=bias_eps[0:1, :]
        )
        ln_p = const.tile([1, M], f32)
        nc.scalar.activation(out=ln_p[:], in_=pps[0:1, :], func=AF.Ln)
        diff = const.tile([1, M], f32)
        nc.vector.tensor_sub(out=diff[:], in0=ln_t[:], in1=ln_p[:])
        losss = const.tile([1, 1], f32)
        nc.vector.tensor_reduce(
            out=losss[:], in_=diff[:], axis=mybir.AxisListType.X, op=OP.add
        )
        nc.sync.dma_start(out=loss_d[:], in_=losss[:])

    # Bacc.finalize() runs the wait-splitting / ldweights / act-table /
    # extended-ISA codegen passes that walrus requires.
    nc.finalize()
    return nc


def _prep_inputs(features: np.ndarray, labels: np.ndarray):
    """Shard + lay out the full inputs for the 8 cores (host marshalling)."""
    bf16 = ml_dtypes.bfloat16
    x = np.ascontiguousarray(features, dtype=np.float32)
    x_bf = x.astype(bf16)
    # chunk-major x^T: xtc[jc, p, dc*128+jj] = x[jc*128+jj, dc*128+p]
    xtc = np.ascontiguousarray(
        x_bf.reshape(NCH, 128, 4, 128).transpose(0, 3, 2, 1)
    ).reshape(NCH, 128, D)
    xnat = x_bf.reshape(NCH, 128, D)
    lab_f = labels.astype(np.float32)
    lab_ch = lab_f.reshape(NCH, 128)
    LC = NCH * YC
    iota_cl = np.broadcast_to(
        (np.arange(YC, dtype=np.float32) - 1.0)[None, None, :], (128, NCH, YC)
    )
    iota_p = (np.arange(128, dtype=np.float32) - 1.0)[:, None]
    in_maps = []
    for c in range(NCORES):
        r = np.roll(np.arange(NCH), -BCH * c)
        labio = np.empty((128, 2 * LC + M + 2), dtype=np.float32)
        labio[:, 0:LC] = iota_cl.reshape(128, LC)
        # labbc[p, t, c'] = labels[jc_t*128 + p]
        labio[:, LC : 2 * LC] = np.repeat(lab_ch[r].T, YC, axis=1)
        labio[:, 2 * LC : 2 * LC + M] = lab_f[c * M : (c + 1) * M][None, :]
        labio_bf = labio.astype(bf16)
        # last two bf16 slots per row hold the raw f32 bits of (p - 1)
        labio_bf.view(np.uint16)[:, 2 * LC + M :] = (
            iota_p.astype("<f4").view(np.uint16).reshape(128, 2)
        )
        in_maps.append(
            {
                "xt": np.ascontiguousarray(xtc[r]),
                "xnat": np.ascontiguousarray(xnat[r]),
                "labio": labio_bf,
            }
        )
    return in_maps


def kernel(features: np.ndarray, labels: np.ndarray) -> np.ndarray:
    from concourse.bass_utils import run_bass_kernel_spmd

    if "nc" not in _CACHE:
        _CACHE["nc"] = _build_bass()
    nc = _CACHE["nc"]
    in_maps = _prep_inputs(features, labels)
    res = run_bass_kernel_spmd(nc, in_maps, list(range(NCORES)))
    total = sum(float(r["loss"][0, 0]) for r in res.results)
    return np.float32(total / B)



# revision 6
# speedup vs baseline: 10.0928x; 10.0928x over previous
"""Trainium2 Bass kernel for supervised contrastive loss (8-core SPMD).

Math (per reference):
    f = x / max(||x||, 1e-12)            row-normalized features  [B, D]
    s = (f f^T) / TEMP                                            [B, B]
    E = exp(s) with diag zeroed
    P_i = sum_{j != i, l_j == l_i} E_ij   (positives)
    T_i = sum_{j != i} E_ij               (positives + negatives)
    loss = mean_i [ log(T_i + EPS) - log(P_i) ]

Distribution: row-block shard with an on-device AllGather. Each core
receives ONLY its own 1024 rows (1 MB bf16, chunk-major x^T layout) plus
a small label table — the host->device tunnel is the bottleneck in this
setup, so input bytes are minimized. On device, each core normalizes its
rows, AllGathers the normalized chunks over NeuronLink, and computes its
[j, m] E^T blocks with j on the partition dim so both masked reductions
are TensorEngine partition-contractions:
    PS1[c', m] = sum_j Y'[j, c'] * E[j, m]     (Y' = one-hot(labels) | ones)
row 0 of PS1 = T_m, and P_m = PS1[l_m + 1, m] (recovered with a one-hot
mask + ones-matmul). Per-core scalar partial losses are summed on host.

SPMD uniformity: every core runs the identical program. Own chunks are
processed from SBUF at iterations 0..7 (with the compile-time diagonal
kill); all 64 gathered chunks are processed uniformly afterwards, with
the 8 own-chunk duplicates neutralized by a sentinel label value in the
shipped table (their one-hot AND ones columns compare to all-zero).
"""

import numpy as np
import ml_dtypes

TEMPERATURE = 0.07
EPS = 1e-8
B = 8192
D = 512
NCORES = 8
M = B // NCORES          # 1024 rows per core
NCH = B // 128           # 64 j-chunks of 128
BCH = M // 128           # 8 chunks in the core's own block
NCLS = 100               # label classes
YC = NCLS + 1            # one-hot columns + ones column
SENT = -1000.0           # label sentinel: matches no class, zeroes Y cols
LIO = NCH + BCH + YC     # labio columns: lab_y | lab_own | iota101

_CACHE = {}


def _build_bass():
    import concourse.bass as bass
    import concourse.bacc as bacc
    import concourse.tile as tile
    from concourse import mybir
    from contextlib import ExitStack

    f32 = mybir.dt.float32
    bf16 = mybir.dt.bfloat16
    AF = mybir.ActivationFunctionType
    OP = mybir.AluOpType

    nc = bacc.Bacc(num_devices=NCORES)

    # ---- I/O ----------------------------------------------------------
    # xt8[k, p, dc*128+jj] = x[own_base + 128k + jj, dc*128+p]
    xt8_d = nc.declare_dram_parameter("xt8", [BCH, 128, D], bf16, isOutput=False)
    # labio[:, 0:64]    lab_y[p, u] = labels[u*128+p], own chunks -> SENT
    # labio[:, 64:72]   lab_own[p, k] = labels[own_base + 128k + p]
    # labio[:, 72:173]  iota101[p, c'] = c' - 1
    labio_d = nc.declare_dram_parameter("labio", [128, LIO], bf16, isOutput=False)
    loss_d = nc.declare_dram_parameter("loss", [1, 1], f32, isOutput=True)

    with ExitStack() as ctx:
        tc = ctx.enter_context(tile.TileContext(nc))
        const = ctx.enter_context(tc.tile_pool(name="const", bufs=1))
        gp = ctx.enter_context(tc.tile_pool(name="gp", bufs=4))
        ep = ctx.enter_context(tc.tile_pool(name="ep", bufs=3))
        psum = ctx.enter_context(tc.tile_pool(name="psum", bufs=3, space="PSUM"))
        accp = ctx.enter_context(tc.tile_pool(name="accp", bufs=1, space="PSUM"))
        dram = ctx.enter_context(tc.tile_pool(name="dram", bufs=1, space="DRAM"))

        # ---- label machinery ------------------------------------------
        labio = const.tile([128, LIO], bf16)
        nc.sync.dma_start(out=labio[:], in_=labio_d[:])
        laby = labio[:, 0:NCH]
        labown = labio[:, NCH : NCH + BCH]
        iota101 = labio[:, NCH + BCH : LIO]
        # is_equal needs an f32 scalar AP; cast the label columns up front
        labf = const.tile([128, NCH + BCH], f32)
        nc.vector.tensor_copy(out=labf[:], in_=labio[:, 0 : NCH + BCH])
        labyf = labf[:, 0:NCH]
        labownf = labf[:, NCH : NCH + BCH]

        # Y for gathered chunks: yg[p, u, c'] = (labels[u*128+p] == c'-1)
        # for c' >= 1; col 0 (the T-sum ones column) = (label != SENT).
        yg = const.tile([128, NCH, YC], bf16)
        nc.vector.tensor_scalar(
            out=yg[:, :, 0:1].rearrange("p u o -> p (u o)"), in0=laby,
            scalar1=SENT, scalar2=None, op0=OP.not_equal,
        )
        for u in range(NCH):
            nc.vector.tensor_scalar(
                out=yg[:, u, 1:YC], in0=iota101[:, 1:YC],
                scalar1=labyf[:, u : u + 1], scalar2=None, op0=OP.is_equal,
            )

        # Y for own chunks (diag handled by affine_select on E instead)
        yo = const.tile([128, BCH, YC], bf16)
        nc.vector.memset(yo[:, :, 0:1], 1.0)
        for k in range(BCH):
            nc.vector.tensor_scalar(
                out=yo[:, k, 1:YC], in0=iota101[:, 1:YC],
                scalar1=labownf[:, k : k + 1], scalar2=None, op0=OP.is_equal,
            )

        # YblkT[c', m] = (labels[own m] == c'-1): per-chunk PE transposes
        # of yo (identity built on device with an affine_select diagonal).
        ident = const.tile([128, 128], bf16)
        nc.vector.memset(ident[:], 1.0)
        nc.gpsimd.affine_select(
            out=ident[:], in_=ident[:], pattern=[[1, 128]],
            compare_op=OP.is_equal, fill=0.0, base=0, channel_multiplier=-1,
        )
        trans_ps = psum.tile([128, M], bf16, tag="sim", name="trans_ps")
        for k in range(BCH):
            nc.tensor.transpose(
                trans_ps[0:YC, k * 128 : (k + 1) * 128], yo[:, k, :], ident[:]
            )
        yblkt = const.tile([128, M], bf16)
        nc.vector.tensor_copy(out=yblkt[0:YC, :], in_=trans_ps[0:YC, :])
        # row 0 is the transposed ones column — must not count T into P
        nc.vector.memset(yblkt[0:1, :], 0.0)

        ones101 = const.tile([128, 1], f32)
        nc.vector.memset(ones101[:], 1.0)
        bias_eps = const.tile([128, 1], f32)
        nc.vector.memset(bias_eps[:], EPS)

        # ---- own rows: load, row norms, normalize ---------------------
        x8 = const.tile([128, BCH, D], bf16)
        nc.sync.dma_start(out=x8[:], in_=xt8_d[:].rearrange("t p f -> p t f"))

        # nsq_row[0, m] = sum_d x[m, d]^2 via DVE square + accumulated
        # ones-matmul partition reductions (4 dc groups x 2 halves).
        ones_bf = const.tile([128, 1], bf16)
        nc.vector.memset(ones_bf[:], 1.0)
        x8sq = const.tile([128, BCH, D], bf16)
        nc.vector.tensor_tensor(out=x8sq[:], in0=x8[:], in1=x8[:], op=OP.mult)
        nsqrow_ps = psum.tile([128, M], f32, tag="sim", name="nsqrow_ps")
        for dc in range(4):
            for h in range(2):
                nc.tensor.matmul(
                    nsqrow_ps[0:1, h * 512 : (h + 1) * 512],
                    lhsT=ones_bf[:, 0:1],
                    rhs=x8sq[:, h * 4 : (h + 1) * 4, dc * 128 : (dc + 1) * 128],
                    start=(dc == 0),
                    stop=(dc == 3),
                )
        # 1/||x_m|| = exp(-0.5*ln(nsq)) on partition 0 of a zeroed tile,
        # broadcast to all partitions with a ones-matmul.
        lnrow = const.tile([1, M], f32)
        nc.scalar.activation(out=lnrow[:], in_=nsqrow_ps[0:1, :], func=AF.Ln)
        rowpad = const.tile([128, M], f32)
        nc.vector.memset(rowpad[:], 0.0)
        nc.scalar.activation(
            out=rowpad[0:1, :], in_=lnrow[:], func=AF.Exp, bias=0.0, scale=-0.5
        )
        ones_f = const.tile([128, 128], f32)
        nc.vector.memset(ones_f[:], 1.0)
        invnbc_ps = psum.tile([128, M], f32, tag="sim", name="invnbc_ps")
        for h in range(2):
            nc.tensor.matmul(
                invnbc_ps[:, h * 512 : (h + 1) * 512],
                lhsT=ones_f[:],
                rhs=rowpad[:, h * 512 : (h + 1) * 512],
                start=True,
                stop=True,
            )
        invnbc = const.tile([128, M], f32)
        nc.vector.tensor_copy(out=invnbc[:], in_=invnbc_ps[:])
        # normalized own-block x^T: xnt[p, dc, m] = x[m, dc*128+p]/||x_m||
        xnt = const.tile([128, 4, M], bf16)
        for dc in range(4):
            nc.vector.tensor_tensor(
                out=xnt[:, dc, :].rearrange("p (t j) -> p t j", j=128),
                in0=x8[:, :, dc * 128 : (dc + 1) * 128],
                in1=invnbc[:].rearrange("p (t j) -> p t j", j=128),
                op=OP.mult,
            )

        # ---- AllGather normalized chunks over NeuronLink --------------
        gin = dram.tile([BCH, 128, D], bf16)
        gout = dram.tile([NCH, 128, D], bf16)
        nc.gpsimd.dma_start(
            gin[:].rearrange("k p (dc jj) -> p dc k jj", jj=128),
            xnt[:, :, :].rearrange("p dc (k jj) -> p dc k jj", jj=128),
        )
        nc.gpsimd.collective_compute(
            "AllGather",
            mybir.AluOpType.bypass,
            replica_groups=[list(range(NCORES))],
            ins=[gin.opt()],
            outs=[gout.opt()],
        )

        # ---- main loop: 8 own chunks (SBUF) + 64 gathered chunks ------
        NT = BCH + NCH
        ps1 = accp.tile([128, M], f32)  # row 0: T; rows 1..100: class sums
        for t in range(NT):
            if t < BCH:
                lhs = None
            else:
                g = gp.tile([128, D], bf16)
                nc.sync.dma_start(out=g[:], in_=gout[t - BCH])
                lhs = g[:]
            ps = psum.tile([128, M], f32, tag="sim")
            for dc in range(4):
                lhsT = (
                    xnt[:, dc, t * 128 : (t + 1) * 128]
                    if t < BCH
                    else lhs[:, dc * 128 : (dc + 1) * 128]
                )
                for h in range(2):
                    nc.tensor.matmul(
                        ps[:, h * 512 : (h + 1) * 512],
                        lhsT=lhsT,
                        rhs=xnt[:, dc, h * 512 : (h + 1) * 512],
                        start=(dc == 0),
                        stop=(dc == 3),
                    )
            e_t = ep.tile([128, M], bf16)
            nc.scalar.activation(
                out=e_t[:], in_=ps[:], func=AF.Exp, scale=float(1.0 / TEMPERATURE)
            )
            if t < BCH:
                # zero the diagonal: kill (p, m) where m - p - 128*t == 0
                nc.gpsimd.affine_select(
                    out=e_t[:], in_=e_t[:], pattern=[[1, M]],
                    compare_op=OP.not_equal, fill=0.0,
                    base=-(t * 128), channel_multiplier=-1,
                )
            yt = yo[:, t, :] if t < BCH else yg[:, t - BCH, :]
            for h in range(2):
                nc.tensor.matmul(
                    ps1[0:YC, h * 512 : (h + 1) * 512],
                    lhsT=yt,
                    rhs=e_t[:, h * 512 : (h + 1) * 512],
                    start=(t == 0),
                    stop=(t == NT - 1),
                )

        # ---- finalize: P via one-hot mask + partition reduce ----------
        maskd = const.tile([128, M], f32)
        nc.vector.tensor_tensor(
            out=maskd[0:YC, :], in0=ps1[0:YC, :], in1=yblkt[0:YC, :], op=OP.mult
        )
        pps = psum.tile([128, M], f32, tag="sim")
        for h in range(2):
            nc.tensor.matmul(
                pps[0:1, h * 512 : (h + 1) * 512],
                lhsT=ones101[0:YC, 0:1],
                rhs=maskd[0:YC, h * 512 : (h + 1) * 512],
                start=True,
                stop=True,
            )
        ln_t = const.tile([1, M], f32)
        nc.scalar.activation(
            out=ln_t[:], in_=ps1[0:1, :], func=AF.Ln, bias=bias_eps[0:1, :]
        )
        ln_p = const.tile([1, M], f32)
        nc.scalar.activation(out=ln_p[:], in_=pps[0:1, :], func=AF.Ln)
        diff = const.tile([1, M], f32)
        nc.vector.tensor_sub(out=diff[:], in0=ln_t[:], in1=ln_p[:])
        losss = const.tile([1, 1], f32)
        nc.vector.tensor_reduce(
            out=losss[:], in_=diff[:], axis=mybir.AxisListType.X, op=OP.add
        )
        nc.sync.dma_start(out=loss_d[:], in_=losss[:])

    nc.finalize()
    return nc


def _prep_inputs(features: np.ndarray, labels: np.ndarray):
    """Shard the full inputs for the 8 cores (host marshalling)."""
    bf16 = ml_dtypes.bfloat16
    x_bf = np.ascontiguousarray(features, dtype=np.float32).astype(bf16)
    # chunk-major x^T: xtc[u, p, dc*128+jj] = x[u*128+jj, dc*128+p]
    xtc = np.ascontiguousarray(
        x_bf.reshape(NCH, 128, 4, 128).transpose(0, 3, 2, 1)
    ).reshape(NCH, 128, D)
    lab_ch = labels.astype(np.float32).reshape(NCH, 128).T  # [p, u]
    iota101 = (np.arange(YC, dtype=np.float32) - 1.0)[None, :]
    in_maps = []
    for c in range(NCORES):
        labio = np.empty((128, LIO), dtype=np.float32)
        labio[:, 0:NCH] = lab_ch
        labio[:, BCH * c : BCH * (c + 1)] = SENT
        labio[:, NCH : NCH + BCH] = lab_ch[:, BCH * c : BCH * (c + 1)]
        labio[:, NCH + BCH : LIO] = iota101
        in_maps.append(
            {
                "xt8": xtc[BCH * c : BCH * (c + 1)],
                "labio": labio.astype(bf16),
            }
        )
    return in_maps


def kernel(features: np.ndarray, labels: np.ndarray) -> np.ndarray:
    from concourse.bass_utils import run_bass_kernel_spmd

    if "nc" not in _CACHE:
        _CACHE["nc"] = _build_bass()
    nc = _CACHE["nc"]
    in_maps = _prep_inputs(features, labels)
    res = run_bass_kernel_spmd(nc, in_maps, list(range(NCORES)))
    total = sum(float(r["loss"][0, 0]) for r in res.results)
    return np.float32(total / B)


# revision 14
# speedup vs baseline: 22.7053x; 2.2496x over previous
"""Trainium2 Bass kernel for supervised contrastive loss (8-core SPMD).

Math (per reference):
    f = x / max(||x||, 1e-12)            row-normalized features  [B, D]
    s = (f f^T) / TEMP                                            [B, B]
    E = exp(s) with diag zeroed
    P_i = sum_{j != i, l_j == l_i} E_ij   (positives)
    T_i = sum_{j != i} E_ij               (positives + negatives)
    loss = mean_i [ log(T_i + EPS) - log(P_i) ]

Distribution: row-block shard with an on-device AllGather. Each core
receives ONLY its own 1024 rows (1 MB bf16, chunk-major x^T layout) plus
a small label table — the host->device tunnel is the bottleneck in this
setup, so input bytes are minimized. On device, each core normalizes its
rows, AllGathers the normalized chunks over NeuronLink, and computes its
[j, m] E^T blocks with j on the partition dim so both masked reductions
are TensorEngine partition-contractions:
    PS1[c', m] = sum_j Y'[j, c'] * E[j, m]     (Y' = one-hot(labels) | ones)
row 0 of PS1 = T_m, and P_m = PS1[l_m + 1, m] (recovered with a one-hot
mask + ones-matmul). Per-core scalar partial losses are summed on host.

SPMD uniformity: every core runs the identical program. Own chunks are
processed from SBUF at iterations 0..7 (with the compile-time diagonal
kill); all 64 gathered chunks are processed uniformly afterwards, with
the 8 own-chunk duplicates neutralized by a sentinel label value in the
shipped table (their one-hot AND ones columns compare to all-zero).
"""

import numpy as np
import ml_dtypes

TEMPERATURE = 0.07
EPS = 1e-8
B = 8192
D = 512
NCORES = 8
M = B // NCORES          # 1024 rows per core
NCH = B // 128           # 64 j-chunks of 128
BCH = M // 128           # 8 chunks in the core's own block
NCLS = 100               # label classes
YC = NCLS + 1            # one-hot columns + ones column
SENT = -1000.0           # label sentinel: matches no class, zeroes Y cols
LIO = NCH + BCH + YC     # labio columns: lab_y | lab_own | iota101

FP8 = True               # ship features as float8_e3m4 (1 B/elem)
_CACHE = {}


def _enable_jax_executable_cache():
    """Persist compiled XLA executables (with the embedded NEFF) so repeat
    run_bass_kernel_spmd calls skip the per-call BIR->NEFF recompile that
    the fresh jit closure in run_bass_via_pjrt otherwise triggers."""
    try:
        import os
        import tempfile
        import jax

        d = os.path.join(tempfile.gettempdir(), "jax_exec_cache")
        os.makedirs(d, exist_ok=True)
        jax.config.update("jax_compilation_cache_dir", d)
        jax.config.update("jax_persistent_cache_min_compile_time_secs", 0)
        jax.config.update("jax_persistent_cache_min_entry_size_bytes", 0)
    except Exception:
        pass


def _build_bass(fp8: bool = FP8, n_gather: int = NCH):
    import concourse.bass as bass
    import concourse.bacc as bacc
    import concourse.tile as tile
    from concourse import mybir
    from contextlib import ExitStack

    f32 = mybir.dt.float32
    bf16 = mybir.dt.bfloat16
    xdt = mybir.dt.float8e3 if fp8 else bf16
    AF = mybir.ActivationFunctionType
    OP = mybir.AluOpType

    nc = bacc.Bacc(num_devices=NCORES)

    # ---- I/O ----------------------------------------------------------
    # xt8[k, p, dc*128+jj] = x[own_base + 128k + jj, dc*128+p]
    xt8_d = nc.declare_dram_parameter("xt8", [BCH, 128, D], xdt, isOutput=False)
    # labio[:, 0:64]    lab_y[p, u] = labels[u*128+p], own chunks -> SENT
    # labio[:, 64:72]   lab_own[p, k] = labels[own_base + 128k + p]
    # labio[:, 72:173]  iota101[p, c'] = c' - 1
    labio_d = nc.declare_dram_parameter("labio", [128, LIO], bf16, isOutput=False)
    loss_d = nc.declare_dram_parameter("loss", [1, 1], f32, isOutput=True)

    with ExitStack() as ctx:
        tc = ctx.enter_context(tile.TileContext(nc))
        const = ctx.enter_context(tc.tile_pool(name="const", bufs=1))
        gp = ctx.enter_context(tc.tile_pool(name="gp", bufs=4))
        ep = ctx.enter_context(tc.tile_pool(name="ep", bufs=3))
        psum = ctx.enter_context(tc.tile_pool(name="psum", bufs=3, space="PSUM"))
        accp = ctx.enter_context(tc.tile_pool(name="accp", bufs=1, space="PSUM"))
        dram = ctx.enter_context(tc.tile_pool(name="dram", bufs=1, space="DRAM"))

        # ---- label machinery ------------------------------------------
        labio = const.tile([128, LIO], bf16)
        nc.sync.dma_start(out=labio[:], in_=labio_d[:])
        laby = labio[:, 0:NCH]
        labown = labio[:, NCH : NCH + BCH]
        iota101 = labio[:, NCH + BCH : LIO]
        # is_equal needs an f32 scalar AP; cast the label columns up front
        labf = const.tile([128, NCH + BCH], f32)
        nc.vector.tensor_copy(out=labf[:], in_=labio[:, 0 : NCH + BCH])
        labyf = labf[:, 0:NCH]
        labownf = labf[:, NCH : NCH + BCH]

        # Y for gathered chunks: yg[p, u, c'] = (labels[u*128+p] == c'-1)
        # for c' >= 1; col 0 (the T-sum ones column) = (label != SENT).
        yg = const.tile([128, NCH, YC], bf16)
        nc.vector.tensor_scalar(
            out=yg[:, :, 0:1].rearrange("p u o -> p (u o)"), in0=laby,
            scalar1=SENT, scalar2=None, op0=OP.not_equal,
        )
        for u in range(NCH):
            nc.vector.tensor_scalar(
                out=yg[:, u, 1:YC], in0=iota101[:, 1:YC],
                scalar1=labyf[:, u : u + 1], scalar2=None, op0=OP.is_equal,
            )

        # Y for own chunks (diag handled by affine_select on E instead)
        yo = const.tile([128, BCH, YC], bf16)
        nc.vector.memset(yo[:, :, 0:1], 1.0)
        for k in range(BCH):
            nc.vector.tensor_scalar(
                out=yo[:, k, 1:YC], in0=iota101[:, 1:YC],
                scalar1=labownf[:, k : k + 1], scalar2=None, op0=OP.is_equal,
            )

        # YblkT[c', m] = (labels[own m] == c'-1): per-chunk PE transposes
        # of yo (identity built on device with an affine_select diagonal).
        ident = const.tile([128, 128], bf16)
        nc.vector.memset(ident[:], 1.0)
        nc.gpsimd.affine_select(
            out=ident[:], in_=ident[:], pattern=[[1, 128]],
            compare_op=OP.is_equal, fill=0.0, base=0, channel_multiplier=-1,
        )
        trans_ps = psum.tile([128, M], bf16, tag="sim", name="trans_ps")
        for k in range(BCH):
            nc.tensor.transpose(
                trans_ps[0:YC, k * 128 : (k + 1) * 128], yo[:, k, :], ident[:]
            )
        yblkt = const.tile([128, M], bf16)
        nc.vector.tensor_copy(out=yblkt[0:YC, :], in_=trans_ps[0:YC, :])
        # row 0 is the transposed ones column — must not count T into P
        nc.vector.memset(yblkt[0:1, :], 0.0)

        ones101 = const.tile([128, 1], f32)
        nc.vector.memset(ones101[:], 1.0)
        bias_eps = const.tile([128, 1], f32)
        nc.vector.memset(bias_eps[:], EPS)

        # ---- own rows: load, row norms, normalize ---------------------
        if fp8:
            x8r = const.tile([128, BCH, D], xdt)
            nc.sync.dma_start(out=x8r[:], in_=xt8_d[:].rearrange("t p f -> p t f"))
            x8 = const.tile([128, BCH, D], bf16)
            nc.vector.tensor_copy(out=x8[:], in_=x8r[:])
        else:
            x8 = const.tile([128, BCH, D], bf16)
            nc.sync.dma_start(out=x8[:], in_=xt8_d[:].rearrange("t p f -> p t f"))

        # nsq_row[0, m] = sum_d x[m, d]^2 via DVE square + accumulated
        # ones-matmul partition reductions (4 dc groups x 2 halves).
        ones_bf = const.tile([128, 1], bf16)
        nc.vector.memset(ones_bf[:], 1.0)
        x8sq = const.tile([128, BCH, D], bf16)
        nc.vector.tensor_tensor(out=x8sq[:], in0=x8[:], in1=x8[:], op=OP.mult)
        nsqrow_ps = psum.tile([128, M], f32, tag="sim", name="nsqrow_ps")
        for dc in range(4):
            for h in range(2):
                nc.tensor.matmul(
                    nsqrow_ps[0:1, h * 512 : (h + 1) * 512],
                    lhsT=ones_bf[:, 0:1],
                    rhs=x8sq[:, h * 4 : (h + 1) * 4, dc * 128 : (dc + 1) * 128],
                    start=(dc == 0),
                    stop=(dc == 3),
                )
        # 1/||x_m|| = exp(-0.5*ln(nsq)) on partition 0 of a zeroed tile,
        # broadcast to all partitions with a ones-matmul.
        lnrow = const.tile([1, M], f32)
        nc.scalar.activation(out=lnrow[:], in_=nsqrow_ps[0:1, :], func=AF.Ln)
        rowpad = const.tile([128, M], f32)
        nc.vector.memset(rowpad[:], 0.0)
        nc.scalar.activation(
            out=rowpad[0:1, :], in_=lnrow[:], func=AF.Exp, bias=0.0, scale=-0.5
        )
        ones_f = const.tile([128, 128], f32)
        nc.vector.memset(ones_f[:], 1.0)
        invnbc_ps = psum.tile([128, M], f32, tag="sim", name="invnbc_ps")
        for h in range(2):
            nc.tensor.matmul(
                invnbc_ps[:, h * 512 : (h + 1) * 512],
                lhsT=ones_f[:],
                rhs=rowpad[:, h * 512 : (h + 1) * 512],
                start=True,
                stop=True,
            )
        invnbc = const.tile([128, M], f32)
        nc.vector.tensor_copy(out=invnbc[:], in_=invnbc_ps[:])
        # normalized own-block x^T: xnt[p, dc, m] = x[m, dc*128+p]/||x_m||
        xnt = const.tile([128, 4, M], bf16)
        for dc in range(4):
            nc.vector.tensor_tensor(
                out=xnt[:, dc, :].rearrange("p (t j) -> p t j", j=128),
                in0=x8[:, :, dc * 128 : (dc + 1) * 128],
                in1=invnbc[:].rearrange("p (t j) -> p t j", j=128),
                op=OP.mult,
            )

        # ---- AllGather normalized chunks over NeuronLink --------------
        gin = dram.tile([BCH, 128, D], bf16)
        gout = dram.tile([NCH, 128, D], bf16)
        nc.gpsimd.dma_start(
            gin[:].rearrange("k p (dc jj) -> p dc k jj", jj=128),
            xnt[:, :, :].rearrange("p dc (k jj) -> p dc k jj", jj=128),
        )
        nc.gpsimd.collective_compute(
            "AllGather",
            mybir.AluOpType.bypass,
            replica_groups=[list(range(NCORES))],
            ins=[gin.opt()],
            outs=[gout.opt()],
        )

        # ---- main loop: 8 own chunks (SBUF) + 64 gathered chunks ------
        NT = BCH + n_gather
        ps1 = accp.tile([128, M], f32)  # row 0: T; rows 1..100: class sums
        for t in range(NT):
            if t < BCH:
                lhs = None
            else:
                g = gp.tile([128, D], bf16)
                nc.sync.dma_start(out=g[:], in_=gout[t - BCH])
                lhs = g[:]
            ps = psum.tile([128, M], f32, tag="sim")
            for dc in range(4):
                lhsT = (
                    xnt[:, dc, t * 128 : (t + 1) * 128]
                    if t < BCH
                    else lhs[:, dc * 128 : (dc + 1) * 128]
                )
                for h in range(2):
                    nc.tensor.matmul(
                        ps[:, h * 512 : (h + 1) * 512],
                        lhsT=lhsT,
                        rhs=xnt[:, dc, h * 512 : (h + 1) * 512],
                        start=(dc == 0),
                        stop=(dc == 3),
                    )
            e_t = ep.tile([128, M], bf16)
            nc.scalar.activation(
                out=e_t[:], in_=ps[:], func=AF.Exp, scale=float(1.0 / TEMPERATURE)
            )
            if t < BCH:
                # zero the diagonal: kill (p, m) where m - p - 128*t == 0
                nc.gpsimd.affine_select(
                    out=e_t[:], in_=e_t[:], pattern=[[1, M]],
                    compare_op=OP.not_equal, fill=0.0,
                    base=-(t * 128), channel_multiplier=-1,
                )
            yt = yo[:, t, :] if t < BCH else yg[:, t - BCH, :]
            for h in range(2):
                nc.tensor.matmul(
                    ps1[0:YC, h * 512 : (h + 1) * 512],
                    lhsT=yt,
                    rhs=e_t[:, h * 512 : (h + 1) * 512],
                    start=(t == 0),
                    stop=(t == NT - 1),
                )

        # ---- finalize: P via one-hot mask + partition reduce ----------
        maskd = const.tile([128, M], f32)
        nc.vector.tensor_tensor(
            out=maskd[0:YC, :], in0=ps1[0:YC, :], in1=yblkt[0:YC, :], op=OP.mult
        )
        pps = psum.tile([128, M], f32, tag="sim")
        for h in range(2):
            nc.tensor.matmul(
                pps[0:1, h * 512 : (h + 1) * 512],
                lhsT=ones101[0:YC, 0:1],
                rhs=maskd[0:YC, h * 512 : (h + 1) * 512],
                start=True,
                stop=True,
            )
        ln_t = const.tile([1, M], f32)
        nc.scalar.activation(
            out=ln_t[:], in_=ps1[0:1, :], func=AF.Ln, bias=bias_eps[0:1, :]
        )
        ln_p = const.tile([1, M], f32)
        nc.scalar.activation(out=ln_p[:], in_=pps[0:1, :], func=AF.Ln)
        diff = const.tile([1, M], f32)
        nc.vector.tensor_sub(out=diff[:], in0=ln_t[:], in1=ln_p[:])
        losss = const.tile([1, 1], f32)
        nc.vector.tensor_reduce(
            out=losss[:], in_=diff[:], axis=mybir.AxisListType.X, op=OP.add
        )
        nc.sync.dma_start(out=loss_d[:], in_=losss[:])

    nc.finalize()
    return nc


def _prep_inputs(features: np.ndarray, labels: np.ndarray, fp8: bool = FP8):
    """Shard the full inputs for the 8 cores (host marshalling)."""
    bf16 = ml_dtypes.bfloat16
    xdt = ml_dtypes.float8_e3m4 if fp8 else bf16
    x_bf = np.ascontiguousarray(features, dtype=np.float32).astype(xdt)
    # chunk-major x^T: xtc[u, p, dc*128+jj] = x[u*128+jj, dc*128+p]
    xtc = np.ascontiguousarray(
        x_bf.reshape(NCH, 128, 4, 128).transpose(0, 3, 2, 1)
    ).reshape(NCH, 128, D)
    del x_bf
    lab_ch = labels.astype(np.float32).reshape(NCH, 128).T  # [p, u]
    iota101 = (np.arange(YC, dtype=np.float32) - 1.0)[None, :]
    in_maps = []
    for c in range(NCORES):
        labio = np.empty((128, LIO), dtype=np.float32)
        labio[:, 0:NCH] = lab_ch
        labio[:, BCH * c : BCH * (c + 1)] = SENT
        labio[:, NCH : NCH + BCH] = lab_ch[:, BCH * c : BCH * (c + 1)]
        labio[:, NCH + BCH : LIO] = iota101
        in_maps.append(
            {
                "xt8": xtc[BCH * c : BCH * (c + 1)],
                "labio": labio.astype(bf16),
            }
        )
    return in_maps


def kernel(features: np.ndarray, labels: np.ndarray) -> np.ndarray:
    from concourse.bass_utils import run_bass_kernel_spmd

    _enable_jax_executable_cache()
    if "nc" not in _CACHE:
        _CACHE["nc"] = _build_bass()
    nc = _CACHE["nc"]
    in_maps = _prep_inputs(features, labels)
    res = run_bass_kernel_spmd(nc, in_maps, list(range(NCORES)))
    total = sum(float(r["loss"][0, 0]) for r in res.results)
    return np.float32(total / B)
